# revision 25
# baseline (speedup 1.0000x reference)
"""Causal multi-head attention (B=2, T=4096, D=1024, H=16, HD=64) on 8 trn2
NeuronCores.

Sharding: core c handles batch b = c//4 and head group g = c%4 (heads
4g..4g+3).  Each core computes qkv projection for its 4 heads, causal
flash-attention in transposed (S^T) layout, and a partial out-projection
(its 256 columns of the hidden dim).  Host sums the 4 partial outputs per
batch and adds the bias terms.

v3: software-pipelined attention i-loop.  The v2 kernel emitted
QK(i), PV(i), QK(i+1), ... in PE program order; since PV(i) waits on
exp(i) (ACT ~1.05us busy) and engines issue in order, every chunk paid
exp+PV+QK serially (~1.9us).  v3 emits QK(i+1) and the filler units
BEFORE PV(i), so the PE computes the next chunk's scores and the
projections while ACT runs exp(i); steady state is ACT-bound at
~1.05us/chunk.  The causal mask moved from a post-exp multiply on pT
(which sat on the exp->PV critical edge) to a pre-exp additive -1e30
mask on the PSUM scores (QK->mask on DVE happens well before exp needs
it).  Normalization multiplies psO (PSUM) directly with the broadcast
reciprocal (PSUM), skipping the intermediate SBUF copy.

Math notes:
  - k-bias kept (cheap), v-bias folded into the host epilogue: softmax
    rows sum to 1, so out += b_v exactly, hence y += b_v @ w_out (+ b_out).
  - softmax computed without max subtraction (scores are O(10) for this
    problem scale; exp stays in fp32 range).
  - softmax denominators come for free as a 65th ones-column in v.
dtypes: q/k path float32r (TF32-like, ~1e-4), P and v bf16, accum fp32.
"""

import ml_dtypes
import numpy as np

import concourse.bass as bass
import concourse.mybir as mybir
import concourse.tile as tile
from concourse import bacc
from concourse.bass_utils import run_bass_kernel_spmd
from concourse.masks import make_upper_triangular

F32 = mybir.dt.float32
F32R = mybir.dt.float32r
BF16 = mybir.dt.bfloat16
AF = mybir.ActivationFunctionType

B, D, H, HD = 2, 1024, 16, 64
NHEADS = 4          # heads per core
SCALE = 1.0 / np.sqrt(HD)


def build(T=4096, reps=1):
    """Build the per-core Bass module. reps>1 wraps the compute in an
    on-device For_i loop (for wall-clock-difference timing)."""
    NJ = T // 512       # tq tiles of 512
    NT = T // 128       # t chunks of 128
    DC = D // 128       # d chunks of 128

    nc = bacc.Bacc("TRN2", target_bir_lowering=False, debug=False, num_devices=8)

    xt_d = nc.dram_tensor("xt", [D, T], BF16, kind="ExternalInput")
    wqk_d = nc.dram_tensor("wqk", [D, 512], BF16, kind="ExternalInput")
    wv_d = nc.dram_tensor("wv", [D, 256], BF16, kind="ExternalInput")
    bqk_d = nc.dram_tensor("bqk", [128, 4], F32, kind="ExternalInput")
    wout_d = nc.dram_tensor("wout", [128, 2, D], BF16, kind="ExternalInput")
    y_d = nc.dram_tensor("y", [T, D], F32, kind="ExternalOutput")

    with tile.TileContext(nc) as tc:
        with (
            tc.tile_pool(name="const", bufs=1) as cp,
            tc.tile_pool(name="persist", bufs=1) as pp,
        ):
            # ---------- constants / weights (outside the timing loop)
            tri32 = cp.tile([128, 128], F32, tag="tri32")
            make_upper_triangular(nc, tri32[:], val=1.0, diag=True)
            # multiplicative causal mask (1 where q >= k, 0 below) applied to
            # pT after exp: with PV lagging 2 chunks behind exp, this DVE op
            # sits in the exp->PV slack instead of the QK->exp feed path, and
            # bf16-in/bf16-out SBUF qualifies for the DVE 2x mode.
            tri = cp.tile([128, 128], BF16, tag="tri")
            nc.vector.tensor_copy(tri[:], tri32[:])

            # ones row on partition 64 (for the denominator broadcast matmul)
            ones65 = cp.tile([65, 64], BF16, tag="ones65")
            nc.vector.memset(ones65[64:65, :], 1.0)

            bqk_sb = cp.tile([128, 4], F32, tag="bqk")
            nc.sync.dma_start(bqk_sb[:], bqk_d[:])
            wqk_sb = cp.tile([128, DC, 512], BF16, tag="wqk")
            nc.sync.dma_start(wqk_sb[:], wqk_d.rearrange("(dc p) c -> p dc c", p=128))
            wv_sb = cp.tile([128, DC, 256], BF16, tag="wv")
            nc.sync.dma_start(wv_sb[:], wv_d.rearrange("(dc p) c -> p dc c", p=128))
            wout_sb = cp.tile([128, 2, D], BF16, tag="wout")
            nc.sync.dma_start(wout_sb[:], wout_d[:])

            # ---------- persistent state
            kT = pp.tile([128, 2, T], BF16, tag="kT")          # [qk-col, pair, t]
            v_sb = pp.tile([128, NT, 4, 65], BF16, tag="v")    # [t%128, tchunk, head, hd+one]
            nc.vector.memset(v_sb[:, :, :, 64:65], 1.0)

            def body():
                with (
                    tc.tile_pool(name="work2", bufs=2) as wp2,
                    tc.tile_pool(name="work3", bufs=3) as wp3,
                    tc.tile_pool(name="outn", bufs=4) as opool,
                    tc.tile_pool(name="norm", bufs=2) as npool,
                    tc.tile_pool(name="yout", bufs=3) as ypool,
                    tc.tile_pool(name="pmisc", bufs=2, space="PSUM") as ps_m,
                    tc.tile_pool(name="pscore", bufs=2, space="PSUM") as ps_s,
                    tc.tile_pool(name="pout", bufs=1, space="PSUM") as ps_o,
                ):
                    # ---- filler units: projection of tile jn (emitted
                    #      interleaved into the previous tile's attention
                    #      i-loop so PE fills its ACT-bound slack and the
                    #      next tile's QK is ready immediately)
                    def load_xT(jn):
                        xT = wp2.tile([128, DC, 512], BF16, tag="xT")
                        nc.sync.dma_start(
                            xT[:],
                            xt_d[:, 512 * jn : 512 * jn + 512].rearrange(
                                "(dc p) t -> p dc t", p=128
                            ),
                        )
                        return xT

                    # filler units are split into sub-closures of <= 4
                    # matmuls so a drained filler never inserts a multi-us
                    # burst between consecutive QK chunks (which would starve
                    # ACT).  PSUM tiles are shared across a unit's subs via a
                    # mutable cell.
                    def proj_qk_subs(jn, cc, xT, qT):
                        cell = {}

                        def s0():
                            pqk = ps_m.tile([128, 512], F32, tag="m")
                            cell["pqk"] = pqk
                            for dc in range(4):
                                nc.tensor.matmul(
                                    pqk[:],
                                    wqk_sb[:, dc, 128 * cc : 128 * (cc + 1)],
                                    xT[:, dc],
                                    start=(dc == 0),
                                    stop=False,
                                    skip_group_check=True,
                                )

                        def s1():
                            pqk = cell["pqk"]
                            for dc in range(4, DC):
                                nc.tensor.matmul(
                                    pqk[:],
                                    wqk_sb[:, dc, 128 * cc : 128 * (cc + 1)],
                                    xT[:, dc],
                                    start=False,
                                    stop=(dc == DC - 1),
                                    skip_group_check=True,
                                )
                            dst = (
                                qT[:, cc]
                                if cc < 2
                                else kT[:, cc - 2, 512 * jn : 512 * jn + 512]
                            )
                            with nc.allow_low_precision(reason="bf16 q/k tiles"):
                                nc.vector.tensor_scalar_add(
                                    dst, pqk[:], bqk_sb[:, cc : cc + 1]
                                )

                        return [s0, s1]

                    def proj_v_subs(jn, ts, xT):
                        cell = {}

                        def s0():
                            pv = ps_m.tile([128, 256], F32, tag="m")
                            cell["pv"] = pv
                            for dc in range(4):
                                nc.tensor.matmul(
                                    pv[:],
                                    xT[:, dc, 128 * ts : 128 * (ts + 1)],
                                    wv_sb[:, dc],
                                    start=(dc == 0),
                                    stop=False,
                                    skip_group_check=True,
                                )

                        def s1():
                            pv = cell["pv"]
                            for dc in range(4, DC):
                                nc.tensor.matmul(
                                    pv[:],
                                    xT[:, dc, 128 * ts : 128 * (ts + 1)],
                                    wv_sb[:, dc],
                                    start=False,
                                    stop=(dc == DC - 1),
                                    skip_group_check=True,
                                )
                            nc.vector.tensor_copy(
                                v_sb[:, 4 * jn + ts, :, 0:64],
                                pv[:].rearrange("p (h c) -> p h c", h=4),
                            )

                        return [s0, s1]

                    def outproj_subs(jp, tt, outN_p):
                        cell = {}

                        def mk(nh):
                            def s():
                                if nh == 0:
                                    ysb = ypool.tile([128, 2, 512], F32, tag="y")
                                    cell["ysb"] = ysb
                                else:
                                    ysb = cell["ysb"]
                                pY = ps_m.tile([128, 512], F32, tag="m")
                                for p in range(2):
                                    nc.tensor.matmul(
                                        pY[:],
                                        outN_p[:, p, 128 * tt : 128 * (tt + 1)],
                                        wout_sb[:, p, 512 * nh : 512 * (nh + 1)],
                                        start=(p == 0),
                                        stop=(p == 1),
                                    )
                                nc.vector.tensor_copy(ysb[:, nh, :], pY[:])
                                if nh == 1:
                                    nc.sync.dma_start(
                                        y_d[
                                            512 * jp + 128 * tt : 512 * jp + 128 * (tt + 1),
                                            :,
                                        ],
                                        ysb[:].rearrange("p a b -> p (a b)"),
                                    )

                            return s

                        return [mk(0), mk(1)]

                    # ---- prologue: projection for tile 0
                    xT0 = load_xT(0)
                    qT_cur = wp2.tile([128, 2, 512], BF16, tag="qTj")
                    for cc in range(4):
                        for s in proj_qk_subs(0, cc, xT0, qT_cur):
                            s()
                    for ts in range(4):
                        for s in proj_v_subs(0, ts, xT0):
                            s()

                    # ---- normalization for a finished (j, hp) segment:
                    #      recip of the ones-row sums (partition 64),
                    #      broadcast down 64 partitions via a K=1 matmul,
                    #      multiply into outN on the PSUM evacuation.
                    def emit_norm(psO_t, outN, hp, hh):
                        psO_h = psO_t[:, 512 * hh : 512 * (hh + 1)]
                        rr = npool.tile([65, 512], BF16, tag="rr")
                        with nc.allow_low_precision(
                            reason="bf16 reciprocal of softmax denom"
                        ):
                            nc.vector.reciprocal(rr[64:65, :], psO_h[64:65, :])
                        pB = ps_m.tile([64, 512], F32, tag="m")
                        nc.tensor.matmul(
                            pB[:],
                            ones65[64:65, :],
                            rr[64:65, :],
                            start=True,
                            stop=True,
                        )
                        pBs = npool.tile([64, 512], F32, tag="pbs")
                        nc.vector.tensor_copy(pBs[:], pB[:])
                        if hh == 0:
                            with nc.allow_low_precision(reason="bf16 attn output"):
                                nc.vector.tensor_tensor(
                                    outN[0:64, hp, :],
                                    psO_h[0:64, :],
                                    pBs[:],
                                    mybir.AluOpType.mult,
                                )
                        else:
                            stage = npool.tile([64, 512], BF16, tag="stage")
                            with nc.allow_low_precision(reason="bf16 attn output"):
                                nc.vector.tensor_tensor(
                                    stage[:],
                                    psO_h[0:64, :],
                                    pBs[:],
                                    mybir.AluOpType.mult,
                                )
                            # odd head of the pair -> partitions 64-127
                            nc.sync.dma_start(outN[64:128, hp, :], stage[:])

                    # ---- flat chunk stream over all (tile j, head pair hp,
                    #      key chunk i), software-pipelined with PV lagging
                    #      2 chunks behind QK/exp.  By the time PV(c-2) is at
                    #      the head of the PE queue its exp has long finished,
                    #      so the PE never idles waiting on ACT; conversely
                    #      QK(c) is emitted 2 chunks ahead so ACT never waits
                    #      on the PE.  Normalization of a segment is deferred
                    #      2 iterations past its last PV so the reciprocal
                    #      chain stays off the PE critical path.
                    chunks = [
                        (j, hp, i)
                        for j in range(NJ)
                        for hp in range(2)
                        for i in range(4 * (j + 1))
                    ]
                    NCH = len(chunks)
                    LAG = 3

                    fillers = []
                    it = 0
                    emitted = 0
                    n_it = 1
                    qTj = None
                    qT_nxt = qT_cur
                    outN_by_j = [None] * NJ
                    pvq = []  # pending (c, pT, dlt, j, hp, i)
                    norm_q = []  # pending (due_c, psO_t, j, hp, hh)
                    psO_cur = None  # psO tile of the segment whose PVs emit

                    for c in range(NCH + LAG + 4):
                        if c < NCH:
                            j, hp, i = chunks[c]
                            if hp == 0 and i == 0:
                                # tile start: rotate qT, new outN + fillers
                                while emitted < len(fillers):
                                    fillers[emitted]()
                                    emitted += 1
                                qTj = qT_nxt
                                qT_nxt = None
                                outN_t = opool.tile([128, 2, 512], BF16, tag="outN")
                                outN_by_j[j] = outN_t
                                fillers = []
                                it = 0
                                emitted = 0
                                n_it = 2 * 4 * (j + 1)
                                if j + 1 < NJ:
                                    xTn = load_xT(j + 1)
                                    qT_nxt = wp2.tile([128, 2, 512], BF16, tag="qTj")
                                    for cc in range(4):
                                        fillers.extend(
                                            proj_qk_subs(j + 1, cc, xTn, qT_nxt)
                                        )
                                    for ts in range(4):
                                        fillers.extend(proj_v_subs(j + 1, ts, xTn))
                                # out-projection of tile j-2 (ready since
                                # early in tile j-1): late tiles have the ACT
                                # slack to absorb it; early tiles are already
                                # PE-bound with projection work.  Tile NJ-1
                                # additionally picks up tile NJ-2's.
                                ops = []
                                if j >= 2:
                                    ops.append(j - 2)
                                if j == NJ - 1:
                                    ops.append(j - 1)
                                for jp in ops:
                                    for tt in range(4):
                                        fillers.extend(
                                            outproj_subs(jp, tt, outN_by_j[jp])
                                        )
                            # ---- QK + mask + exp for chunk c
                            dlt = 128 * i - 512 * j
                            dlt = dlt if dlt > 0 else 0
                            pS = ps_s.tile([128, 1024], F32, tag="ps")
                            for hh in range(2):
                                nc.tensor.matmul(
                                    pS[:, 512 * hh + dlt : 512 * (hh + 1)],
                                    kT[64 * hh : 64 * (hh + 1), hp, 128 * i : 128 * (i + 1)],
                                    qTj[64 * hh : 64 * (hh + 1), hp, dlt:512],
                                    start=True,
                                    stop=True,
                                )
                            pT = wp3.tile([128, 2, 512], BF16, tag="pT", bufs=8)
                            pSv = pS[:].rearrange("p (h w) -> p h w", h=2)
                            nc.scalar.activation(
                                pT[:, :, dlt:512], pSv[:, :, dlt:512], AF.Exp, scale=SCALE
                            )
                            if i >= 4 * j:  # diagonal block: causal 0/1 mask
                                for hh in range(2):
                                    nc.vector.tensor_tensor(
                                        pT[:, hh, dlt : dlt + 128],
                                        pT[:, hh, dlt : dlt + 128],
                                        tri[:],
                                        mybir.AluOpType.mult,
                                    )
                            pvq.append((c, pT, dlt, j, hp, i))
                            it += 1
                            want = len(fillers) * it // n_it
                            while emitted < want:
                                fillers[emitted]()
                                emitted += 1
                        # ---- deferred normalizations that are due (one hh
                        #      per entry; staggered across iterations)
                        while norm_q and norm_q[0][0] <= c:
                            _, psO_d, j_d, hp_d, hh_d = norm_q.pop(0)
                            emit_norm(psO_d, outN_by_j[j_d], hp_d, hh_d)
                        # ---- PV for chunk c-LAG.  A segment's first PV
                        #      reallocates the single psO buffer, so it must
                        #      not emit until the previous segment's norms
                        #      (psO readers) have been emitted.
                        pops = 0
                        while (
                            pvq
                            and pops < 2
                            and pvq[0][0] <= c - LAG
                            and (pvq[0][5] != 0 or not norm_q)
                        ):
                            pops += 1
                            _, pTp, dltp, pj, php, pi = pvq.pop(0)
                            pnchunk = 4 * (pj + 1)
                            if pi == 0:
                                psO_cur = ps_o.tile([65, 1024], F32, tag="po")
                            psO = [psO_cur[:, 0:512], psO_cur[:, 512:1024]]
                            for hh in range(2):
                                nc.tensor.matmul(
                                    psO[hh][0:65, dltp:512],
                                    v_sb[:, pi, 2 * php + hh, :],
                                    pTp[:, hh, dltp:512],
                                    start=(pi == 0),
                                    stop=(pi == pnchunk - 1),
                                    skip_group_check=True,
                                )
                            if pi == pnchunk - 1:
                                norm_q.append((c + 1, psO_cur, pj, php, 0))
                                norm_q.append((c + 2, psO_cur, pj, php, 1))

                    assert not pvq, f"undrained PVs: {len(pvq)}"
                    # ---- epilogue: remaining norms, fillers, out-projection
                    while norm_q:
                        _, psO_d, j_d, hp_d, hh_d = norm_q.pop(0)
                        emit_norm(psO_d, outN_by_j[j_d], hp_d, hh_d)
                    while emitted < len(fillers):
                        fillers[emitted]()
                        emitted += 1
                    for tt in range(4):
                        for s in outproj_subs(NJ - 1, tt, outN_by_j[NJ - 1]):
                            s()

            if reps == 1:
                body()
            else:
                with tc.For_i(0, reps, 1):
                    body()

    nc.compile()
    return nc


def shard_inputs(x, w_qkv, b_qkv, w_out, T):
    """Build the 8 per-core input maps (core c: batch c//4, head group c%4)."""
    x = np.asarray(x, dtype=np.float32)
    w_qkv = np.asarray(w_qkv, dtype=np.float32)
    b_qkv = np.asarray(b_qkv, dtype=np.float32)
    w_out = np.asarray(w_out, dtype=np.float32)
    in_maps = []
    for c in range(8):
        b, g = c // 4, c % 4
        qcols = slice(4 * g * 64, (4 * g + 4) * 64)
        kcols = slice(D + 4 * g * 64, D + (4 * g + 4) * 64)
        vcols = slice(2 * D + 4 * g * 64, 2 * D + (4 * g + 4) * 64)
        wqk = np.concatenate([w_qkv[:, qcols], w_qkv[:, kcols]], axis=1)  # [D, 512]
        wv = np.ascontiguousarray(w_qkv[:, vcols])  # [D, 256]
        bqk = np.concatenate([b_qkv[qcols], b_qkv[kcols]]).reshape(4, 128).T  # [128,4]
        # [hidden 256] -> [pair p, in-pair ip, hd], stacked so partitions
        # 0-63 = head 2p, 64-127 = head 2p+1
        w4 = w_out[256 * g : 256 * (g + 1), :].reshape(2, 2, 64, D)
        wout = np.ascontiguousarray(w4.transpose(1, 2, 0, 3).reshape(128, 2, D))
        bf16 = ml_dtypes.bfloat16
        in_maps.append(
            {
                "xt": np.ascontiguousarray(x[b, :T].T).astype(bf16),
                "wqk": np.ascontiguousarray(wqk).astype(bf16),
                "wv": wv.astype(bf16),
                "bqk": np.ascontiguousarray(bqk),
                "wout": wout.astype(bf16),
            }
        )
    return in_maps


def assemble_output(results, b_qkv, b_out, w_out, T):
    b_qkv = np.asarray(b_qkv, dtype=np.float32)
    b_out = np.asarray(b_out, dtype=np.float32)
    w_out = np.asarray(w_out, dtype=np.float32)
    extra = b_out + b_qkv[2 * D :] @ w_out  # v-bias folds through softmax
    y = np.zeros((B, T, D), dtype=np.float32)
    for c in range(8):
        y[c // 4] += results[c]["y"]
    y += extra[None, None, :]
    return y


_cache = {}


def kernel(x, w_qkv, b_qkv, w_out, b_out):
    x = np.asarray(x, dtype=np.float32)
    T = x.shape[1]
    if T not in _cache:
        _cache[T] = build(T=T, reps=1)
    nc = _cache[T]
    in_maps = shard_inputs(x, w_qkv, b_qkv, w_out, T)
    for _attempt in range(3):
        res = run_bass_kernel_spmd(nc, in_maps, core_ids=list(range(8)), trace=False)
        y = assemble_output(res.results, b_qkv, b_out, w_out, T)
        if np.isfinite(y).all():  # guard against transient device flakes
            return y
    return y



# revision 27
# speedup vs baseline: 1.0077x; 1.0077x over previous
"""Causal multi-head attention (B=2, T=4096, D=1024, H=16, HD=64) on 8 trn2
NeuronCores.

Sharding: core c handles batch b = c//4 and head group g = c%4 (heads
4g..4g+3).  Each core computes qkv projection for its 4 heads, causal
flash-attention in transposed (S^T) layout, and a partial out-projection
(its 256 columns of the hidden dim).  Host sums the 4 partial outputs per
batch and adds the bias terms.

v7: flat software-pipelined chunk stream.  All (tile j, head-pair hp,
key-chunk i) iterations form one continuous pipeline; PV lags 3 chunks
behind QK/exp (so PV never blocks the PE queue on ACT) and each
segment's normalization is deferred and staggered per-head into the
next segment's iterations.  The v2 kernel emitted QK(i), PV(i),
QK(i+1), ... in PE program order; since PV(i) waits on exp(i) (ACT
~1.05us busy) and engines issue in order, every chunk paid exp+PV+QK
serially (~1.9us).  Filler units (next tile's qkv projection, tile
j-2's out-projection) are split into <=4-matmul sub-closures and paced
one per chunk so they never insert a multi-us burst between
consecutive QK chunks (which would starve ACT).  Out-projections are
assigned to tile j+2: the late (big) tiles have ACT slack to absorb
them, while early tiles are already PE-bound with projection work.
The causal 0/1 tri-mask multiplies pT after exp (bf16 SBUF, DVE 2x
eligible); with the PV lag it sits in the exp->PV slack, off the
QK->exp feed path.

Math notes:
  - k-bias kept (cheap), v-bias folded into the host epilogue: softmax
    rows sum to 1, so out += b_v exactly, hence y += b_v @ w_out (+ b_out).
  - softmax computed without max subtraction (scores are O(10) for this
    problem scale; exp stays in fp32 range).
  - softmax denominators come for free as a 65th ones-column in v.
dtypes: q/k path float32r (TF32-like, ~1e-4), P and v bf16, accum fp32.
"""

import shutil as _shutil

import ml_dtypes
import numpy as np

# The PJRT-level NEFF cache keys on an HLO hash that does NOT include the
# bass BIR embedded in the custom call, so a NEFF compiled from a *different*
# kernel version (same shapes) would be silently reused.  Drop any stale
# cache once at import so every compile in this process is self-consistent.
_shutil.rmtree("/root/.neuron-compile-cache", ignore_errors=True)

import concourse.bass as bass
import concourse.mybir as mybir
import concourse.tile as tile
from concourse import bacc
from concourse.bass_utils import run_bass_kernel_spmd
from concourse.masks import make_upper_triangular

F32 = mybir.dt.float32
F32R = mybir.dt.float32r
BF16 = mybir.dt.bfloat16
AF = mybir.ActivationFunctionType

B, D, H, HD = 2, 1024, 16, 64
NHEADS = 4          # heads per core
SCALE = 1.0 / np.sqrt(HD)


def build(T=4096, reps=1):
    """Build the per-core Bass module. reps>1 wraps the compute in an
    on-device For_i loop (for wall-clock-difference timing)."""
    NJ = T // 512       # tq tiles of 512
    NT = T // 128       # t chunks of 128
    DC = D // 128       # d chunks of 128

    nc = bacc.Bacc("TRN2", target_bir_lowering=False, debug=False, num_devices=8)

    xt_d = nc.dram_tensor("xt", [D, T], BF16, kind="ExternalInput")
    wqk_d = nc.dram_tensor("wqk", [D, 512], BF16, kind="ExternalInput")
    wv_d = nc.dram_tensor("wv", [D, 256], BF16, kind="ExternalInput")
    bqk_d = nc.dram_tensor("bqk", [128, 4], F32, kind="ExternalInput")
    wout_d = nc.dram_tensor("wout", [128, 2, D], BF16, kind="ExternalInput")
    y_d = nc.dram_tensor("y", [T, D], F32, kind="ExternalOutput")

    with tile.TileContext(nc) as tc:
        with (
            tc.tile_pool(name="const", bufs=1) as cp,
            tc.tile_pool(name="persist", bufs=1) as pp,
        ):
            # ---------- constants / weights (outside the timing loop)
            tri32 = cp.tile([128, 128], F32, tag="tri32")
            make_upper_triangular(nc, tri32[:], val=1.0, diag=True)
            # multiplicative causal mask (1 where q >= k, 0 below) applied to
            # pT after exp: with PV lagging 2 chunks behind exp, this DVE op
            # sits in the exp->PV slack instead of the QK->exp feed path, and
            # bf16-in/bf16-out SBUF qualifies for the DVE 2x mode.
            tri = cp.tile([128, 128], BF16, tag="tri")
            nc.vector.tensor_copy(tri[:], tri32[:])

            # ones row on partition 64 (for the denominator broadcast matmul)
            ones65 = cp.tile([65, 64], BF16, tag="ones65")
            nc.vector.memset(ones65[64:65, :], 1.0)

            bqk_sb = cp.tile([128, 4], F32, tag="bqk")
            nc.sync.dma_start(bqk_sb[:], bqk_d[:])
            wqk_sb = cp.tile([128, DC, 512], BF16, tag="wqk")
            nc.sync.dma_start(wqk_sb[:], wqk_d.rearrange("(dc p) c -> p dc c", p=128))
            wv_sb = cp.tile([128, DC, 256], BF16, tag="wv")
            nc.sync.dma_start(wv_sb[:], wv_d.rearrange("(dc p) c -> p dc c", p=128))
            wout_sb = cp.tile([128, 2, D], BF16, tag="wout")
            nc.sync.dma_start(wout_sb[:], wout_d[:])

            # ---------- persistent state
            kT = pp.tile([128, 2, T], BF16, tag="kT")          # [qk-col, pair, t]
            v_sb = pp.tile([128, NT, 4, 65], BF16, tag="v")    # [t%128, tchunk, head, hd+one]
            nc.vector.memset(v_sb[:, :, :, 64:65], 1.0)

            def body():
                with (
                    tc.tile_pool(name="work2", bufs=2) as wp2,
                    tc.tile_pool(name="work3", bufs=3) as wp3,
                    tc.tile_pool(name="outn", bufs=4) as opool,
                    tc.tile_pool(name="norm", bufs=2) as npool,
                    tc.tile_pool(name="yout", bufs=3) as ypool,
                    tc.tile_pool(name="pmisc", bufs=2, space="PSUM") as ps_m,
                    tc.tile_pool(name="pscore", bufs=2, space="PSUM") as ps_s,
                    tc.tile_pool(name="pout", bufs=1, space="PSUM") as ps_o,
                ):
                    # ---- filler units: projection of tile jn (emitted
                    #      interleaved into the previous tile's attention
                    #      i-loop so PE fills its ACT-bound slack and the
                    #      next tile's QK is ready immediately)
                    def load_xT(jn):
                        xT = wp2.tile([128, DC, 512], BF16, tag="xT")
                        nc.sync.dma_start(
                            xT[:],
                            xt_d[:, 512 * jn : 512 * jn + 512].rearrange(
                                "(dc p) t -> p dc t", p=128
                            ),
                        )
                        return xT

                    # filler units are split into sub-closures of <= 4
                    # matmuls so a drained filler never inserts a multi-us
                    # burst between consecutive QK chunks (which would starve
                    # ACT).  PSUM tiles are shared across a unit's subs via a
                    # mutable cell.
                    def proj_qk_subs(jn, cc, xT, qT):
                        cell = {}

                        def s0():
                            pqk = ps_m.tile([128, 512], F32, tag="m")
                            cell["pqk"] = pqk
                            for dc in range(4):
                                nc.tensor.matmul(
                                    pqk[:],
                                    wqk_sb[:, dc, 128 * cc : 128 * (cc + 1)],
                                    xT[:, dc],
                                    start=(dc == 0),
                                    stop=False,
                                    skip_group_check=True,
                                )

                        def s1():
                            pqk = cell["pqk"]
                            for dc in range(4, DC):
                                nc.tensor.matmul(
                                    pqk[:],
                                    wqk_sb[:, dc, 128 * cc : 128 * (cc + 1)],
                                    xT[:, dc],
                                    start=False,
                                    stop=(dc == DC - 1),
                                    skip_group_check=True,
                                )
                            dst = (
                                qT[:, cc]
                                if cc < 2
                                else kT[:, cc - 2, 512 * jn : 512 * jn + 512]
                            )
                            with nc.allow_low_precision(reason="bf16 q/k tiles"):
                                nc.vector.tensor_scalar_add(
                                    dst, pqk[:], bqk_sb[:, cc : cc + 1]
                                )

                        return [s0, s1]

                    def proj_v_subs(jn, ts, xT):
                        cell = {}

                        def s0():
                            pv = ps_m.tile([128, 256], F32, tag="m")
                            cell["pv"] = pv
                            for dc in range(4):
                                nc.tensor.matmul(
                                    pv[:],
                                    xT[:, dc, 128 * ts : 128 * (ts + 1)],
                                    wv_sb[:, dc],
                                    start=(dc == 0),
                                    stop=False,
                                    skip_group_check=True,
                                )

                        def s1():
                            pv = cell["pv"]
                            for dc in range(4, DC):
                                nc.tensor.matmul(
                                    pv[:],
                                    xT[:, dc, 128 * ts : 128 * (ts + 1)],
                                    wv_sb[:, dc],
                                    start=False,
                                    stop=(dc == DC - 1),
                                    skip_group_check=True,
                                )
                            nc.vector.tensor_copy(
                                v_sb[:, 4 * jn + ts, :, 0:64],
                                pv[:].rearrange("p (h c) -> p h c", h=4),
                            )

                        return [s0, s1]

                    def outproj_subs(jp, tt, outN_p):
                        cell = {}

                        def mk(nh):
                            def s():
                                if nh == 0:
                                    ysb = ypool.tile([128, 2, 512], F32, tag="y")
                                    cell["ysb"] = ysb
                                else:
                                    ysb = cell["ysb"]
                                pY = ps_m.tile([128, 512], F32, tag="m")
                                for p in range(2):
                                    nc.tensor.matmul(
                                        pY[:],
                                        outN_p[:, p, 128 * tt : 128 * (tt + 1)],
                                        wout_sb[:, p, 512 * nh : 512 * (nh + 1)],
                                        start=(p == 0),
                                        stop=(p == 1),
                                    )
                                nc.vector.tensor_copy(ysb[:, nh, :], pY[:])
                                if nh == 1:
                                    nc.sync.dma_start(
                                        y_d[
                                            512 * jp + 128 * tt : 512 * jp + 128 * (tt + 1),
                                            :,
                                        ],
                                        ysb[:].rearrange("p a b -> p (a b)"),
                                    )

                            return s

                        return [mk(0), mk(1)]

                    # ---- prologue: projection for tile 0
                    xT0 = load_xT(0)
                    qT_cur = wp2.tile([128, 2, 512], BF16, tag="qTj")
                    for cc in range(4):
                        for s in proj_qk_subs(0, cc, xT0, qT_cur):
                            s()
                    for ts in range(4):
                        for s in proj_v_subs(0, ts, xT0):
                            s()

                    # ---- normalization for a finished (j, hp) segment:
                    #      recip of the ones-row sums (partition 64),
                    #      broadcast down 64 partitions via a K=1 matmul,
                    #      multiply into outN on the PSUM evacuation.
                    def emit_norm(psO_t, outN, hp, hh):
                        psO_h = psO_t[:, 512 * hh : 512 * (hh + 1)]
                        rr = npool.tile([65, 512], BF16, tag="rr")
                        with nc.allow_low_precision(
                            reason="bf16 reciprocal of softmax denom"
                        ):
                            nc.vector.reciprocal(rr[64:65, :], psO_h[64:65, :])
                        pB = ps_m.tile([64, 512], F32, tag="m")
                        nc.tensor.matmul(
                            pB[:],
                            ones65[64:65, :],
                            rr[64:65, :],
                            start=True,
                            stop=True,
                        )
                        pBs = npool.tile([64, 512], F32, tag="pbs")
                        nc.vector.tensor_copy(pBs[:], pB[:])
                        if hh == 0:
                            with nc.allow_low_precision(reason="bf16 attn output"):
                                nc.vector.tensor_tensor(
                                    outN[0:64, hp, :],
                                    psO_h[0:64, :],
                                    pBs[:],
                                    mybir.AluOpType.mult,
                                )
                        else:
                            stage = npool.tile([64, 512], BF16, tag="stage")
                            with nc.allow_low_precision(reason="bf16 attn output"):
                                nc.vector.tensor_tensor(
                                    stage[:],
                                    psO_h[0:64, :],
                                    pBs[:],
                                    mybir.AluOpType.mult,
                                )
                            # odd head of the pair -> partitions 64-127
                            nc.sync.dma_start(outN[64:128, hp, :], stage[:])

                    # ---- flat chunk stream over all (tile j, head pair hp,
                    #      key chunk i), software-pipelined with PV lagging
                    #      2 chunks behind QK/exp.  By the time PV(c-2) is at
                    #      the head of the PE queue its exp has long finished,
                    #      so the PE never idles waiting on ACT; conversely
                    #      QK(c) is emitted 2 chunks ahead so ACT never waits
                    #      on the PE.  Normalization of a segment is deferred
                    #      2 iterations past its last PV so the reciprocal
                    #      chain stays off the PE critical path.
                    chunks = [
                        (j, hp, i)
                        for j in range(NJ)
                        for hp in range(2)
                        for i in range(4 * (j + 1))
                    ]
                    NCH = len(chunks)
                    LAG = 3

                    fillers = []
                    it = 0
                    emitted = 0
                    n_it = 1
                    qTj = None
                    qT_nxt = qT_cur
                    outN_by_j = [None] * NJ
                    pvq = []  # pending (c, pT, dlt, j, hp, i)
                    norm_q = []  # pending (due_c, psO_t, j, hp, hh)
                    psO_cur = None  # psO tile of the segment whose PVs emit

                    for c in range(NCH + LAG + 4):
                        if c < NCH:
                            j, hp, i = chunks[c]
                            if hp == 0 and i == 0:
                                # tile start: rotate qT, new outN + fillers
                                while emitted < len(fillers):
                                    fillers[emitted]()
                                    emitted += 1
                                qTj = qT_nxt
                                qT_nxt = None
                                outN_t = opool.tile([128, 2, 512], BF16, tag="outN")
                                outN_by_j[j] = outN_t
                                fillers = []
                                it = 0
                                emitted = 0
                                n_it = 2 * 4 * (j + 1)
                                if j + 1 < NJ:
                                    xTn = load_xT(j + 1)
                                    qT_nxt = wp2.tile([128, 2, 512], BF16, tag="qTj")
                                    for cc in range(4):
                                        fillers.extend(
                                            proj_qk_subs(j + 1, cc, xTn, qT_nxt)
                                        )
                                    for ts in range(4):
                                        fillers.extend(proj_v_subs(j + 1, ts, xTn))
                                # out-projection of tile j-2 (ready since
                                # early in tile j-1): late tiles have the ACT
                                # slack to absorb it; early tiles are already
                                # PE-bound with projection work.  Tile NJ-1
                                # additionally picks up tile NJ-2's.
                                ops = []
                                if j >= 2:
                                    ops.append(j - 2)
                                if j == NJ - 1:
                                    ops.append(j - 1)
                                for jp in ops:
                                    for tt in range(4):
                                        fillers.extend(
                                            outproj_subs(jp, tt, outN_by_j[jp])
                                        )
                            # ---- QK + mask + exp for chunk c
                            dlt = 128 * i - 512 * j
                            dlt = dlt if dlt > 0 else 0
                            pS = ps_s.tile([128, 1024], F32, tag="ps")
                            for hh in range(2):
                                nc.tensor.matmul(
                                    pS[:, 512 * hh + dlt : 512 * (hh + 1)],
                                    kT[64 * hh : 64 * (hh + 1), hp, 128 * i : 128 * (i + 1)],
                                    qTj[64 * hh : 64 * (hh + 1), hp, dlt:512],
                                    start=True,
                                    stop=True,
                                )
                            pT = wp3.tile([128, 2, 512], BF16, tag="pT", bufs=8)
                            pSv = pS[:].rearrange("p (h w) -> p h w", h=2)
                            nc.scalar.activation(
                                pT[:, :, dlt:512], pSv[:, :, dlt:512], AF.Exp, scale=SCALE
                            )
                            if i >= 4 * j:  # diagonal block: causal 0/1 mask
                                for hh in range(2):
                                    nc.vector.tensor_tensor(
                                        pT[:, hh, dlt : dlt + 128],
                                        pT[:, hh, dlt : dlt + 128],
                                        tri[:],
                                        mybir.AluOpType.mult,
                                    )
                            pvq.append((c, pT, dlt, j, hp, i))
                            it += 1
                            want = len(fillers) * it // n_it
                            while emitted < want:
                                fillers[emitted]()
                                emitted += 1
                        # ---- deferred normalizations that are due (one hh
                        #      per entry; staggered across iterations)
                        while norm_q and norm_q[0][0] <= c:
                            _, psO_d, j_d, hp_d, hh_d = norm_q.pop(0)
                            emit_norm(psO_d, outN_by_j[j_d], hp_d, hh_d)
                        # ---- PV for chunk c-LAG.  A segment's first PV
                        #      reallocates the single psO buffer, so it must
                        #      not emit until the previous segment's norms
                        #      (psO readers) have been emitted.
                        pops = 0
                        while (
                            pvq
                            and pops < 2
                            and pvq[0][0] <= c - LAG
                            and (pvq[0][5] != 0 or not norm_q)
                        ):
                            pops += 1
                            _, pTp, dltp, pj, php, pi = pvq.pop(0)
                            pnchunk = 4 * (pj + 1)
                            if pi == 0:
                                psO_cur = ps_o.tile([65, 1024], F32, tag="po")
                            psO = [psO_cur[:, 0:512], psO_cur[:, 512:1024]]
                            for hh in range(2):
                                nc.tensor.matmul(
                                    psO[hh][0:65, dltp:512],
                                    v_sb[:, pi, 2 * php + hh, :],
                                    pTp[:, hh, dltp:512],
                                    start=(pi == 0),
                                    stop=(pi == pnchunk - 1),
                                    skip_group_check=True,
                                )
                            if pi == pnchunk - 1:
                                norm_q.append((c + 1, psO_cur, pj, php, 0))
                                norm_q.append((c + 2, psO_cur, pj, php, 1))

                    assert not pvq, f"undrained PVs: {len(pvq)}"
                    # ---- epilogue: remaining norms, fillers, out-projection
                    while norm_q:
                        _, psO_d, j_d, hp_d, hh_d = norm_q.pop(0)
                        emit_norm(psO_d, outN_by_j[j_d], hp_d, hh_d)
                    while emitted < len(fillers):
                        fillers[emitted]()
                        emitted += 1
                    for tt in range(4):
                        for s in outproj_subs(NJ - 1, tt, outN_by_j[NJ - 1]):
                            s()

            if reps == 1:
                body()
            else:
                with tc.For_i(0, reps, 1):
                    body()

    nc.compile()
    return nc


def shard_inputs(x, w_qkv, b_qkv, w_out, T):
    """Build the 8 per-core input maps (core c: batch c//4, head group c%4)."""
    x = np.asarray(x, dtype=np.float32)
    w_qkv = np.asarray(w_qkv, dtype=np.float32)
    b_qkv = np.asarray(b_qkv, dtype=np.float32)
    w_out = np.asarray(w_out, dtype=np.float32)
    in_maps = []
    for c in range(8):
        b, g = c // 4, c % 4
        qcols = slice(4 * g * 64, (4 * g + 4) * 64)
        kcols = slice(D + 4 * g * 64, D + (4 * g + 4) * 64)
        vcols = slice(2 * D + 4 * g * 64, 2 * D + (4 * g + 4) * 64)
        wqk = np.concatenate([w_qkv[:, qcols], w_qkv[:, kcols]], axis=1)  # [D, 512]
        wv = np.ascontiguousarray(w_qkv[:, vcols])  # [D, 256]
        bqk = np.concatenate([b_qkv[qcols], b_qkv[kcols]]).reshape(4, 128).T  # [128,4]
        # [hidden 256] -> [pair p, in-pair ip, hd], stacked so partitions
        # 0-63 = head 2p, 64-127 = head 2p+1
        w4 = w_out[256 * g : 256 * (g + 1), :].reshape(2, 2, 64, D)
        wout = np.ascontiguousarray(w4.transpose(1, 2, 0, 3).reshape(128, 2, D))
        bf16 = ml_dtypes.bfloat16
        in_maps.append(
            {
                "xt": np.ascontiguousarray(x[b, :T].T).astype(bf16),
                "wqk": np.ascontiguousarray(wqk).astype(bf16),
                "wv": wv.astype(bf16),
                "bqk": np.ascontiguousarray(bqk),
                "wout": wout.astype(bf16),
            }
        )
    return in_maps


def assemble_output(results, b_qkv, b_out, w_out, T):
    b_qkv = np.asarray(b_qkv, dtype=np.float32)
    b_out = np.asarray(b_out, dtype=np.float32)
    w_out = np.asarray(w_out, dtype=np.float32)
    extra = b_out + b_qkv[2 * D :] @ w_out  # v-bias folds through softmax
    y = np.zeros((B, T, D), dtype=np.float32)
    for c in range(8):
        y[c // 4] += results[c]["y"]
    y += extra[None, None, :]
    return y


_cache = {}


def kernel(x, w_qkv, b_qkv, w_out, b_out):
    x = np.asarray(x, dtype=np.float32)
    T = x.shape[1]
    if T not in _cache:
        _cache[T] = build(T=T, reps=1)
    nc = _cache[T]
    in_maps = shard_inputs(x, w_qkv, b_qkv, w_out, T)
    for _attempt in range(3):
        res = run_bass_kernel_spmd(nc, in_maps, core_ids=list(range(8)), trace=False)
        y = assemble_output(res.results, b_qkv, b_out, w_out, T)
        if np.isfinite(y).all():  # guard against transient device flakes
            return y
    return y

